# revision 14
# baseline (speedup 1.0000x reference)
"""Trainium2 Bass kernel for nn_BaseRuleLearner (D-packed redesign).

Math (reference):
  scores[b,i,p] = sum_v UM[b,i,v,perm[p,v]] + sum_{n,m} BM[b,i,n,m,perm[p,n],perm[p,m]]
  out = softmax_i(min_p scores) @ one_hot([0,0,1,1])

Key restructure vs the old kernel: transpose-pair packing. For each
unordered object pair u={j<k}, the stage-1 rhs stacks Bf[b,j,k,:] (rows
0-63) and Bf[b,k,j,:] (rows 64-127); a full-column weight computes
  D[{n,m},(j,k)] = Bf[j,k]·rb[n,m] + Bf[k,j]·rb[m,n]
in ONE contraction (both orderings of the rule pair fused), so the
stage-2 gather matrix needs only 192 k-rows per i (28 pairs×6 + 4
lp×6 merged unary+diagonal) instead of 456 — stage-2 is 32 matmuls
(2 k-chunks × 4i × 4bt) instead of 64, and evac/assembly volume
halves.

Pipeline (per core, 512 batch):
  input DMA: xb [128,(32 slots)(512 b)] in 4×1MB chunks on the sync
  HWDGE ring; w/gm/xu + 3 assembly DMAs on the scalar HWDGE ring.
  stage-1: per slot one MM [24 out rows] into a shared psum bank at
  24-row offsets (5 slots/bank, 7 banks); unary slots accumulate two
  MMs (ru then diag rb).
  evac: 7 ACT copies psum→sg bf16 (one per bank).
  assembly: 3 DMAs total regroup sg [(u i oc),(bank b)] into per-i
  contiguous k-rows qtA[120, (i b)] / qtB[72, (i b)].
  stage-2: per (bt,i): 2 accumulating MMs (qtA/G0, qtB/G1) → sc[128,336].
  final: DVE min-reduce per (bt,i), softmax per bt, one output DMA.
"""

import itertools
import numpy as np

B, O, E = 4096, 8, 64
I, V = 4, 3
P = 336
N_CORES = 8
BC = B // N_CORES            # 512 batch per core
NBT = BC // 128              # 4 b-tiles per core
NSLOT = 32                   # 28 unordered pairs + 4 diag/unary slots
NBANK = 8                    # stage-1 psum banks (4 slots each, 32-aligned)
PAIRS = [(j, k) for j in range(O) for k in range(j + 1, O)]
NM3 = [(0, 1), (0, 2), (1, 2)]
RQ = NSLOT * 8               # 256 stage-2 k-rows per i (8 per slot, 2 pad)
R0 = 128                     # chunk rows: slots 0-15 / 16-31

_CACHED = {}


def _build_g():
    """G[r=slot*6+o*3+c, p]: pair slots u<28 gather D[{n,m},ordered];
    slots 28+d gather merged unary+diag terms for l = 2d+o."""
    perm = np.array(list(itertools.permutations(range(O), V)), dtype=np.int32)
    pidx = {jk: u for u, jk in enumerate(PAIRS)}
    # qt row layout per half: r = bkh*32 + su*8 + oc for slot
    # u = half*16 + bkh*4 + su, oc = o*3 + c.
    def row(u, oc):
        return (u % 16) // 4 * 32 + (u % 4) * 8 + oc

    packed = np.zeros((128, 2 * P), np.float32)
    for p in range(P):
        pp = perm[p]
        for nmi in range(3):
            n, m = NM3[nmi]
            a, b2 = pp[n], pp[m]
            u, o = (pidx[(a, b2)], 0) if a < b2 else (pidx[(b2, a)], 1)
            packed[row(u, o * 3 + nmi), (u // 16) * P + p] = 1.0
        for v in range(V):
            l = pp[v]
            u = 28 + l // 2
            packed[row(u, (l % 2) * 3 + v), (u // 16) * P + p] = 1.0
    return packed


def _build_module():
    import concourse.tile as tile
    from concourse import bacc, mybir

    FP = mybir.dt.float32
    BF = mybir.dt.bfloat16
    X = mybir.AxisListType.X
    nc = bacc.Bacc("TRN2", target_bir_lowering=False, debug=False)

    xb = nc.dram_tensor("xb", [128, NSLOT * BC], BF, kind="ExternalInput")
    xu = nc.dram_tensor("xu", [128, 4 * BC], BF, kind="ExternalInput")
    w = nc.dram_tensor("w", [128, 96], BF, kind="ExternalInput")
    gm = nc.dram_tensor("gm", [128, 2 * P], BF, kind="ExternalInput")
    out = nc.dram_tensor("out", [BC, 4], FP, kind="ExternalOutput")

    with tile.TileContext(nc) as tc:
        with (
            tc.tile_pool(name="wpool", bufs=1) as wpool,
            tc.tile_pool(name="xpool", bufs=1) as xpool,
            tc.tile_pool(name="sgpool", bufs=1) as sgpool,
            tc.tile_pool(name="qpool", bufs=1) as qpool,
            tc.tile_pool(name="mpool", bufs=2) as mpool,
            tc.tile_pool(name="psw", bufs=1, space="PSUM") as psw,
            tc.tile_pool(name="ps1", bufs=3, space="PSUM") as ps1,
            tc.tile_pool(name="pss", bufs=4, space="PSUM") as pss,
        ):
            w_sb = wpool.tile([128, 96], BF, tag="w")
            nc.scalar.dma_start(w_sb[:], w.ap()[:])
            g_sb = wpool.tile([128, 2 * P], BF, tag="g")
            nc.scalar.dma_start(g_sb[:], gm.ap()[:])
            xu_sb = wpool.tile([128, 4 * BC], BF, tag="xu")
            nc.scalar.dma_start(xu_sb[:], xu.ap()[:])
            xt = []
            for gidx in range(4):
                t = xpool.tile([128, 8 * BC], BF, tag=f"x{gidx}", name=f"x{gidx}")
                nc.sync.dma_start(
                    t[:], xb.ap()[:, gidx * 8 * BC:(gidx + 1) * 8 * BC]
                )
                xt.append(t)

            # PE warmup: junk matmuls on a memset tile (no DMA dependency)
            # span the HAM activity window so real matmuls run at 2.4GHz.
            wz = wpool.tile([128, BC], BF, tag="wz")
            nc.gpsimd.memset(wz[:], 0.0)
            warm = psw.tile([128, BC], FP, tag="warm")
            for wi in range(11):
                nc.tensor.matmul(
                    warm[:], wz[:, 0:128], wz[:], start=True, stop=True,
                )

            sg = sgpool.tile([128, NBANK * BC], BF, tag="sg")
            qtA = qpool.tile([128, I * BC], BF, tag="qtA")
            qtB = qpool.tile([128, I * BC], BF, tag="qtB")

            # ---- stage 1 + evac, bank by bank (4 slots @ 32-row steps) ----
            for bk in range(NBANK):
                slots = range(4 * bk, 4 * bk + 4)
                pb = ps1.tile([128, BC], FP, tag="pb")
                for s in slots:
                    row = (s % 4) * 32
                    dst = pb[row:row + 32, :]
                    rhs = xt[s // 8][:, (s % 8) * BC:(s % 8 + 1) * BC]
                    if s < 28:
                        nc.tensor.matmul(
                            dst, w_sb[:, 0:32], rhs, start=True, stop=True,
                            tile_position=(0, row),
                        )
                    else:
                        d = s - 28
                        nc.tensor.matmul(
                            dst, w_sb[:, 32:64],
                            xu_sb[:, d * BC:(d + 1) * BC],
                            start=True, stop=False, tile_position=(0, row),
                        )
                        nc.tensor.matmul(
                            dst, w_sb[:, 64:96], rhs, start=False, stop=True,
                            tile_position=(0, row),
                        )
                nc.scalar.copy(sg[:, bk * BC:(bk + 1) * BC], pb[:])
                # assembly: psum rows q = s*4+i stream as (s, i, b) into
                # the per-i k-row layout r = bkh*32 + s(u*8+oc).
                qt = qtA if bk < 4 else qtB
                dstv = (
                    qt[(bk % 4) * 32:(bk % 4) * 32 + 32, :]
                    .rearrange("p (i b) -> p i b", i=4)
                )
                nc.scalar.dma_start(dstv, sg[:, bk * BC:(bk + 1) * BC])

            # ---- stage 2 + min + softmax ----
            fin = mpool.tile([128, 4 * NBT], FP, tag="fin", bufs=1)
            for bt in range(NBT):
                merged = mpool.tile([128, 4], FP, tag="m")
                for i in range(I):
                    sc = pss.tile([128, 512], FP, tag="sc")
                    col = i * BC + bt * 128
                    nc.tensor.matmul(
                        sc[:, 0:P], qtA[:, col:col + 128],
                        g_sb[:, 0:P], start=True, stop=False,
                    )
                    nc.tensor.matmul(
                        sc[:, 0:P], qtB[:, col:col + 128],
                        g_sb[:, P:2 * P], start=False, stop=True,
                    )
                    nc.vector.tensor_reduce(
                        merged[:, i:i + 1], sc[:, 0:P], axis=X,
                        op=mybir.AluOpType.min,
                    )
                mx = mpool.tile([128, 1], FP, tag="mx")
                nc.vector.tensor_reduce(
                    mx[:], merged[:], axis=X, op=mybir.AluOpType.max
                )
                sh = mpool.tile([128, 4], FP, tag="sh")
                nc.vector.tensor_scalar_sub(sh[:], merged[:], mx[:])
                ex = mpool.tile([128, 4], FP, tag="ex")
                sm = mpool.tile([128, 1], FP, tag="sm")
                nc.scalar.activation(
                    ex[:], sh[:], mybir.ActivationFunctionType.Exp, accum_out=sm[:]
                )
                rc = mpool.tile([128, 1], FP, tag="rc")
                nc.vector.reciprocal(rc[:], sm[:])
                pr = mpool.tile([128, 4], FP, tag="pr")
                nc.vector.tensor_scalar_mul(pr[:], ex[:], rc[:])
                pr3 = pr[:].rearrange("p (a b) -> p a b", b=2)
                nc.vector.tensor_add(
                    fin[:, bt * 4:bt * 4 + 2], pr3[:, :, 0], pr3[:, :, 1]
                )
                nc.vector.memset(fin[:, bt * 4 + 2:bt * 4 + 4], 0.0)
            outv = out.ap().rearrange("(a p) m -> p a m", p=128)
            nc.scalar.dma_start(outv, fin[:].rearrange("p (a m) -> p a m", a=NBT))

    nc.compile()
    return nc


def _get_module():
    if "nc" not in _CACHED:
        _CACHED["nc"] = _build_module()
    return _CACHED["nc"]


def _host_inputs(unary_feats, binary_feats, rule_unary, rule_binary):
    import ml_dtypes

    bf16 = ml_dtypes.bfloat16
    uf = np.asarray(unary_feats, dtype=np.float32).astype(bf16)
    bf = np.asarray(binary_feats, dtype=np.float32).astype(bf16)
    ru = np.asarray(rule_unary, dtype=np.float32)
    rb = np.asarray(rule_binary, dtype=np.float32)

    w = np.zeros((128, 96), np.float32)
    for i in range(I):
        for o in range(2):
            for nmi, (n, m) in enumerate(NM3):
                c = (o * 3 + nmi) * 4 + i
                w[0:64, c] = rb[i, n, m] if o == 0 else rb[i, m, n]
                w[64:128, c] = rb[i, m, n] if o == 0 else rb[i, n, m]
            for v in range(V):
                w[o * 64:(o + 1) * 64, 32 + (o * 3 + v) * 4 + i] = ru[i, v]
                w[o * 64:(o + 1) * 64, 64 + (o * 3 + v) * 4 + i] = rb[i, v, v]
    w = w.astype(bf16)
    g = _build_g().astype(bf16)

    ju = np.array([pr[0] for pr in PAIRS])
    ku = np.array([pr[1] for pr in PAIRS])
    jd = np.arange(O).reshape(4, 2)

    in_maps = []
    for c in range(N_CORES):
        bfc = bf[c * BC:(c + 1) * BC]                    # [BC, O, O, E]
        ufc = uf[c * BC:(c + 1) * BC]                    # [BC, O, E]
        xb = np.empty((2, E, NSLOT, BC), bf16)
        xb[0, :, 0:28] = bfc[:, ju, ku].transpose(2, 1, 0)
        xb[1, :, 0:28] = bfc[:, ku, ju].transpose(2, 1, 0)
        dg = bfc[:, np.arange(O), np.arange(O)]          # [BC, O, E]
        xb[0, :, 28:32] = dg[:, jd[:, 0]].transpose(2, 1, 0)
        xb[1, :, 28:32] = dg[:, jd[:, 1]].transpose(2, 1, 0)
        xuc = np.ascontiguousarray(
            ufc.reshape(BC, 4, 2, E).transpose(2, 3, 1, 0)
        )                                                # [2, E, 4, BC]
        in_maps.append({
            "xb": np.ascontiguousarray(xb).reshape(128, NSLOT * BC),
            "xu": xuc.reshape(128, 4 * BC),
            "w": w, "gm": g,
        })
    return in_maps


TRACE = False  # set True (e.g. from test.py) to capture an NTFF profile


def kernel(unary_feats, binary_feats, rule_unary, rule_binary):
    from concourse.bass_utils import run_bass_kernel_spmd

    nc = _get_module()
    in_maps = _host_inputs(unary_feats, binary_feats, rule_unary, rule_binary)
    res = run_bass_kernel_spmd(
        nc, in_maps, core_ids=list(range(N_CORES)), trace=TRACE
    )
    _CACHED["last_results"] = res
    return np.concatenate(
        [res.results[c]["out"] for c in range(N_CORES)], axis=0
    )


# revision 16
# speedup vs baseline: 1.0812x; 1.0812x over previous
"""Trainium2 Bass kernel for nn_BaseRuleLearner (D-packed redesign).

Math (reference):
  scores[b,i,p] = sum_v UM[b,i,v,perm[p,v]] + sum_{n,m} BM[b,i,n,m,perm[p,n],perm[p,m]]
  out = softmax_i(min_p scores) @ one_hot([0,0,1,1])

Key restructure vs the old kernel: transpose-pair packing. For each
unordered object pair u={j<k}, the stage-1 rhs stacks Bf[b,j,k,:] (rows
0-63) and Bf[b,k,j,:] (rows 64-127); a full-column weight computes
  D[{n,m},(j,k)] = Bf[j,k]·rb[n,m] + Bf[k,j]·rb[m,n]
in ONE contraction (both orderings of the rule pair fused), so the
stage-2 gather matrix needs only 192 k-rows per i (28 pairs×6 + 4
lp×6 merged unary+diagonal) instead of 456 — stage-2 is 32 matmuls
(2 k-chunks × 4i × 4bt) instead of 64, and evac/assembly volume
halves.

Pipeline (per core, 512 batch):
  input DMA: xb [128,(32 slots)(512 b)] in 4×1MB chunks on the sync
  HWDGE ring; w/gm/xu + 3 assembly DMAs on the scalar HWDGE ring.
  stage-1: per slot one MM [24 out rows] into a shared psum bank at
  24-row offsets (5 slots/bank, 7 banks); unary slots accumulate two
  MMs (ru then diag rb).
  evac: 7 ACT copies psum→sg bf16 (one per bank).
  assembly: 3 DMAs total regroup sg [(u i oc),(bank b)] into per-i
  contiguous k-rows qtA[120, (i b)] / qtB[72, (i b)].
  stage-2: per (bt,i): 2 accumulating MMs (qtA/G0, qtB/G1) → sc[128,336].
  final: DVE min-reduce per (bt,i), softmax per bt, one output DMA.
"""

import itertools
import numpy as np

B, O, E = 4096, 8, 64
I, V = 4, 3
P = 336
N_CORES = 8
BC = B // N_CORES            # 512 batch per core
NBT = BC // 128              # 4 b-tiles per core
NSLOT = 32                   # 28 unordered pairs + 4 diag/unary slots
NBANK = 8                    # stage-1 psum banks (4 slots each, 32-aligned)
PAIRS = [(j, k) for j in range(O) for k in range(j + 1, O)]
NM3 = [(0, 1), (0, 2), (1, 2)]
RQ = NSLOT * 8               # 256 stage-2 k-rows per i (8 per slot, 2 pad)
R0 = 128                     # chunk rows: slots 0-15 / 16-31

_CACHED = {}


def _build_g():
    """G[r=slot*6+o*3+c, p]: pair slots u<28 gather D[{n,m},ordered];
    slots 28+d gather merged unary+diag terms for l = 2d+o."""
    perm = np.array(list(itertools.permutations(range(O), V)), dtype=np.int32)
    pidx = {jk: u for u, jk in enumerate(PAIRS)}
    # qt row layout per half: r = bkh*32 + su*8 + oc for slot
    # u = half*16 + bkh*4 + su, oc = o*3 + c.
    def row(u, oc):
        return (u % 16) // 4 * 32 + (u % 4) * 8 + oc

    packed = np.zeros((128, 2 * P), np.float32)
    for p in range(P):
        pp = perm[p]
        for nmi in range(3):
            n, m = NM3[nmi]
            a, b2 = pp[n], pp[m]
            u, o = (pidx[(a, b2)], 0) if a < b2 else (pidx[(b2, a)], 1)
            packed[row(u, o * 3 + nmi), (u // 16) * P + p] = 1.0
        for v in range(V):
            l = pp[v]
            u = 28 + l // 2
            packed[row(u, (l % 2) * 3 + v), (u // 16) * P + p] = 1.0
    return packed


def _build_module():
    import concourse.tile as tile
    from concourse import bacc, mybir

    FP = mybir.dt.float32
    BF = mybir.dt.bfloat16
    X = mybir.AxisListType.X
    nc = bacc.Bacc("TRN2", target_bir_lowering=False, debug=False)

    xb = nc.dram_tensor("xb", [128, NSLOT * BC], BF, kind="ExternalInput")
    xu = nc.dram_tensor("xu", [128, 4 * BC], BF, kind="ExternalInput")
    w = nc.dram_tensor("w", [128, 96], BF, kind="ExternalInput")
    gm = nc.dram_tensor("gm", [128, 2 * P], BF, kind="ExternalInput")
    out = nc.dram_tensor("out", [BC, 4], FP, kind="ExternalOutput")

    with tile.TileContext(nc) as tc:
        with (
            tc.tile_pool(name="wpool", bufs=1) as wpool,
            tc.tile_pool(name="xpool", bufs=1) as xpool,
            tc.tile_pool(name="sgpool", bufs=1) as sgpool,
            tc.tile_pool(name="qpool", bufs=1) as qpool,
            tc.tile_pool(name="mpool", bufs=2) as mpool,
            tc.tile_pool(name="ps1", bufs=4, space="PSUM") as ps1,
            tc.tile_pool(name="pss", bufs=4, space="PSUM") as pss,
        ):
            w_sb = wpool.tile([128, 96], BF, tag="w")
            nc.scalar.dma_start(w_sb[:], w.ap()[:])
            g_sb = wpool.tile([128, 2 * P], BF, tag="g")
            nc.scalar.dma_start(g_sb[:], gm.ap()[:])
            xu_sb = wpool.tile([128, 4 * BC], BF, tag="xu")
            nc.scalar.dma_start(xu_sb[:], xu.ap()[:])
            xt = []
            for gidx in range(4):
                t = xpool.tile([128, 8 * BC], BF, tag=f"x{gidx}", name=f"x{gidx}")
                nc.sync.dma_start(
                    t[:], xb.ap()[:, gidx * 8 * BC:(gidx + 1) * 8 * BC]
                )
                xt.append(t)

            sg = sgpool.tile([128, NBANK * BC], BF, tag="sg")
            qtA = qpool.tile([128, I * BC], BF, tag="qtA")
            qtB = qpool.tile([128, I * BC], BF, tag="qtB")
            # zero tile for HAM-filler matmuls (see below)
            wz = wpool.tile([128, BC], BF, tag="wz")
            nc.gpsimd.memset(wz[:], 0.0)

            # ---- stage 1 + evac, bank by bank (4 slots @ 32-row steps) ----
            for bk in range(NBANK):
                slots = range(4 * bk, 4 * bk + 4)
                pb = ps1.tile([128, BC], FP, tag="pb")
                for s in slots:
                    row = (s % 4) * 32
                    dst = pb[row:row + 32, :]
                    rhs = xt[s // 8][:, (s % 8) * BC:(s % 8 + 1) * BC]
                    if s < 28:
                        nc.tensor.matmul(
                            dst, w_sb[:, 0:32], rhs, start=True, stop=True,
                            tile_position=(0, row),
                        )
                    else:
                        d = s - 28
                        nc.tensor.matmul(
                            dst, w_sb[:, 32:64],
                            xu_sb[:, d * BC:(d + 1) * BC],
                            start=True, stop=False, tile_position=(0, row),
                        )
                        nc.tensor.matmul(
                            dst, w_sb[:, 64:96], rhs, start=False, stop=True,
                            tile_position=(0, row),
                        )
                nc.scalar.copy(sg[:, bk * BC:(bk + 1) * BC], pb[:])
                # two filler matmuls per bank keep the PE busy through the
                # DMA-paced gaps so HAM holds the 2.4GHz clock into stage 2.
                if bk < 7:
                    junk = ps1.tile([128, BC], FP, tag="pb")
                    for _ in range(2):
                        nc.tensor.matmul(
                            junk[:], wz[:, 0:128], wz[:],
                            start=True, stop=True,
                        )
                # assembly: psum rows q = s*4+i stream as (s, i, b) into
                # the per-i k-row layout r = bkh*32 + s(u*8+oc).
                qt = qtA if bk < 4 else qtB
                dstv = (
                    qt[(bk % 4) * 32:(bk % 4) * 32 + 32, :]
                    .rearrange("p (i b) -> p i b", i=4)
                )
                nc.scalar.dma_start(dstv, sg[:, bk * BC:(bk + 1) * BC])

            # ---- stage 2 + min + softmax ----
            fin = mpool.tile([128, 4 * NBT], FP, tag="fin", bufs=1)
            for bt in range(NBT):
                merged = mpool.tile([128, 4], FP, tag="m")
                for i in range(I):
                    sc = pss.tile([128, 512], FP, tag="sc")
                    col = i * BC + bt * 128
                    nc.tensor.matmul(
                        sc[:, 0:P], qtA[:, col:col + 128],
                        g_sb[:, 0:P], start=True, stop=False,
                    )
                    nc.tensor.matmul(
                        sc[:, 0:P], qtB[:, col:col + 128],
                        g_sb[:, P:2 * P], start=False, stop=True,
                    )
                    nc.vector.tensor_reduce(
                        merged[:, i:i + 1], sc[:, 0:P], axis=X,
                        op=mybir.AluOpType.min,
                    )
                mx = mpool.tile([128, 1], FP, tag="mx")
                nc.vector.tensor_reduce(
                    mx[:], merged[:], axis=X, op=mybir.AluOpType.max
                )
                sh = mpool.tile([128, 4], FP, tag="sh")
                nc.vector.tensor_scalar_sub(sh[:], merged[:], mx[:])
                ex = mpool.tile([128, 4], FP, tag="ex")
                sm = mpool.tile([128, 1], FP, tag="sm")
                nc.scalar.activation(
                    ex[:], sh[:], mybir.ActivationFunctionType.Exp, accum_out=sm[:]
                )
                rc = mpool.tile([128, 1], FP, tag="rc")
                nc.vector.reciprocal(rc[:], sm[:])
                pr = mpool.tile([128, 4], FP, tag="pr")
                nc.vector.tensor_scalar_mul(pr[:], ex[:], rc[:])
                pr3 = pr[:].rearrange("p (a b) -> p a b", b=2)
                nc.vector.tensor_add(
                    fin[:, bt * 4:bt * 4 + 2], pr3[:, :, 0], pr3[:, :, 1]
                )
                nc.vector.memset(fin[:, bt * 4 + 2:bt * 4 + 4], 0.0)
            outv = out.ap().rearrange("(a p) m -> p a m", p=128)
            nc.scalar.dma_start(outv, fin[:].rearrange("p (a m) -> p a m", a=NBT))

    nc.compile()
    return nc


def _get_module():
    if "nc" not in _CACHED:
        _CACHED["nc"] = _build_module()
    return _CACHED["nc"]


def _host_inputs(unary_feats, binary_feats, rule_unary, rule_binary):
    import ml_dtypes

    bf16 = ml_dtypes.bfloat16
    uf = np.asarray(unary_feats, dtype=np.float32).astype(bf16)
    bf = np.asarray(binary_feats, dtype=np.float32).astype(bf16)
    ru = np.asarray(rule_unary, dtype=np.float32)
    rb = np.asarray(rule_binary, dtype=np.float32)

    w = np.zeros((128, 96), np.float32)
    for i in range(I):
        for o in range(2):
            for nmi, (n, m) in enumerate(NM3):
                c = (o * 3 + nmi) * 4 + i
                w[0:64, c] = rb[i, n, m] if o == 0 else rb[i, m, n]
                w[64:128, c] = rb[i, m, n] if o == 0 else rb[i, n, m]
            for v in range(V):
                w[o * 64:(o + 1) * 64, 32 + (o * 3 + v) * 4 + i] = ru[i, v]
                w[o * 64:(o + 1) * 64, 64 + (o * 3 + v) * 4 + i] = rb[i, v, v]
    w = w.astype(bf16)
    g = _build_g().astype(bf16)

    ju = np.array([pr[0] for pr in PAIRS])
    ku = np.array([pr[1] for pr in PAIRS])
    jd = np.arange(O).reshape(4, 2)

    in_maps = []
    for c in range(N_CORES):
        bfc = bf[c * BC:(c + 1) * BC]                    # [BC, O, O, E]
        ufc = uf[c * BC:(c + 1) * BC]                    # [BC, O, E]
        xb = np.empty((2, E, NSLOT, BC), bf16)
        xb[0, :, 0:28] = bfc[:, ju, ku].transpose(2, 1, 0)
        xb[1, :, 0:28] = bfc[:, ku, ju].transpose(2, 1, 0)
        dg = bfc[:, np.arange(O), np.arange(O)]          # [BC, O, E]
        xb[0, :, 28:32] = dg[:, jd[:, 0]].transpose(2, 1, 0)
        xb[1, :, 28:32] = dg[:, jd[:, 1]].transpose(2, 1, 0)
        xuc = np.ascontiguousarray(
            ufc.reshape(BC, 4, 2, E).transpose(2, 3, 1, 0)
        )                                                # [2, E, 4, BC]
        in_maps.append({
            "xb": np.ascontiguousarray(xb).reshape(128, NSLOT * BC),
            "xu": xuc.reshape(128, 4 * BC),
            "w": w, "gm": g,
        })
    return in_maps


TRACE = False  # set True (e.g. from test.py) to capture an NTFF profile


def kernel(unary_feats, binary_feats, rule_unary, rule_binary):
    from concourse.bass_utils import run_bass_kernel_spmd

    nc = _get_module()
    in_maps = _host_inputs(unary_feats, binary_feats, rule_unary, rule_binary)
    res = run_bass_kernel_spmd(
        nc, in_maps, core_ids=list(range(N_CORES)), trace=TRACE
    )
    _CACHED["last_results"] = res
    return np.concatenate(
        [res.results[c]["out"] for c in range(N_CORES)], axis=0
    )
